# revision 11
# baseline (speedup 1.0000x reference)
"""Trainium2 Bass kernel for nn_ActorNetwork (Euler-integrated 3-layer MLP).

reference semantics:
    s0 = concat(x, z)                       # [B, 24]
    5x Euler steps: s += 0.2 * MLP(s)       # MLP: 24 ->2048 relu ->2048 relu ->24
    traj = [s0, s1..s5] -> [B, 6, 24], force dims (15:) clamped to [-2, 2]
    returns (traj, final_state, force)

Strategy: pure data parallel over 8 NeuronCores (8192 samples each). All
activations are kept transposed ([feature, batch]) so the natural weight
layout is directly the matmul stationary operand and no transposes are
needed anywhere. Matmuls run as float32r (full-rate fp32 on the PE for
free dims >= 256). Layer biases + ReLU fold into ScalarE activations;
the Euler update is a VectorE add into PSUM followed by a ScalarE
round-copy back into the resident state tile.
"""

import sys

sys.path.insert(0, "/opt/trn_rl_repo")

import numpy as np
import concourse.bass as bass
import concourse.tile as tile
from concourse import bacc, masks, mybir
from concourse.bass_utils import run_bass_kernel_spmd

F32 = mybir.dt.float32
F32R = mybir.dt.float32r
AF = mybir.ActivationFunctionType

N_CORES = 8
BATCH = 65536
B_SHARD = BATCH // N_CORES  # 8192
IN_DIM = 15
FORCE_DIM = 9
D = IN_DIM + FORCE_DIM  # 24
H = 2048
MC = H // 128  # 16 hidden chunks
N_STEPS = 5
DT = 0.2
BT = 512  # batch tile (matmul moving free dim)
NBT = B_SHARD // BT  # 16
GROUP = 8  # batch tiles per resident state group
NG = NBT // GROUP  # 2


def build(b_shard=B_SHARD):
    nbt = b_shard // BT
    ngroups = max(1, nbt // GROUP)
    group = nbt // ngroups

    nc = bacc.Bacc("TRN2", target_bir_lowering=False, debug=False)
    sT_d = nc.dram_tensor("sT", [D, b_shard], F32R, kind="ExternalInput")
    W1_d = nc.dram_tensor("W1", [D, H], F32R, kind="ExternalInput")
    W2_d = nc.dram_tensor("W2", [H, H], F32R, kind="ExternalInput")
    W3_d = nc.dram_tensor("W3", [H, D], F32R, kind="ExternalInput")  # pre-scaled by DT
    b1_d = nc.dram_tensor("b1", [H], F32, kind="ExternalInput")
    b2_d = nc.dram_tensor("b2", [H], F32, kind="ExternalInput")
    b3_d = nc.dram_tensor("b3", [D], F32, kind="ExternalInput")  # pre-scaled by DT
    out_d = nc.dram_tensor("out", [N_STEPS, D, b_shard], F32, kind="ExternalOutput")

    with tile.TileContext(nc) as tc:
        with (
            tc.tile_pool(name="wpool", bufs=1) as wpool,
            tc.tile_pool(name="state", bufs=1) as spool,
            tc.tile_pool(name="h1p", bufs=1) as h1p,
            tc.tile_pool(name="h2p", bufs=10) as h2p,
            tc.tile_pool(name="ps12", bufs=6, space="PSUM") as ps12,
            tc.tile_pool(name="ps3", bufs=2, space="PSUM") as ps3,
        ):
            # --- weights / biases, resident for the whole kernel ---
            w1 = wpool.tile([D, H], F32R, tag="w1")
            nc.sync.dma_start(w1[:], W1_d.ap())
            w2 = []
            for kt in range(MC):
                t = wpool.tile([128, H], F32R, tag=f"w2_{kt}")
                nc.sync.dma_start(t[:], W2_d.ap()[kt * 128 : (kt + 1) * 128, :])
                w2.append(t)
            w3 = []
            for kt in range(MC):
                t = wpool.tile([128, D], F32R, tag=f"w3_{kt}")
                nc.sync.dma_start(t[:], W3_d.ap()[kt * 128 : (kt + 1) * 128, :])
                w3.append(t)
            b1s = wpool.tile([128, MC], F32, tag="b1s")
            nc.sync.dma_start(b1s[:], b1_d.ap().rearrange("(t p) -> p t", p=128))
            b2s = wpool.tile([128, MC], F32, tag="b2s")
            nc.sync.dma_start(b2s[:], b2_d.ap().rearrange("(t p) -> p t", p=128))
            b3s = wpool.tile([D, 1], F32, tag="b3s")
            nc.sync.dma_start(b3s[:], b3_d.ap().rearrange("(p o) -> p o", o=1))
            # 24x24 identity (f32r) - folds the Euler "+ s" into the L3
            # PSUM accumulation as one tiny matmul
            ident_f = wpool.tile([D, D], F32, tag="ident_f")
            masks.make_identity(nc, ident_f[:])
            ident = wpool.tile([D, D], F32R, tag="ident")
            nc.scalar.activation(ident[:], ident_f[:], AF.Identity, bias=0.0)

            h1 = h1p.tile([128, MC * BT], F32R, tag="h1")

            for g in range(ngroups):
                g0 = g * group
                # resident rounded state for this group's batch tiles
                scr = spool.tile([D, group * BT], F32R, tag="scr")
                nc.sync.dma_start(
                    scr[:], sT_d.ap()[:, g0 * BT : (g0 + group) * BT]
                )
                # pending: the previous batch tile's layer-3 tail (8
                # matmuls) + Euler epilogue, deferred into the next tile's
                # layer-1 phase so the PE has matmul work while the ReLU
                # (PSUM->SBUF) pipeline catches up.
                pending = None

                def flush_pending():
                    p3p, hbs, scp, outp = pending
                    for i in range(8):
                        nc.tensor.matmul(
                            p3p[:],
                            w3[8 + i][:],
                            hbs[i][:],
                            start=False,
                            stop=(i == 7),
                            skip_group_check=True,
                        )
                    # Euler epilogue: p3 already holds s + h2@(DT*W3);
                    # add DT*b3, round to f32r, store the new state
                    nc.scalar.activation(scp, p3p[:], AF.Identity, bias=b3s[:])
                    nc.sync.dma_start(outp, scp.bitcast(F32))

                for step in range(N_STEPS):
                    for bt in range(group):
                        sc = scr[:, bt * BT : (bt + 1) * BT]
                        # ---- layer 1: h1[mt] = relu(W1[:,mt].T @ s + b1[mt])
                        # bias+relu alternates ScalarE/VectorE so the L1
                        # epilogue keeps pace with the PE
                        for mt in range(MC):
                            p1 = ps12.tile([128, BT], F32, tag="p12")
                            nc.tensor.matmul(
                                p1[:],
                                w1[:, mt * 128 : (mt + 1) * 128],
                                sc,
                                start=True,
                                stop=True,
                            )
                            if pending is not None and mt == 5:
                                flush_pending()
                                pending = None
                            h1c = h1[:, mt * BT : (mt + 1) * BT]
                            if mt % 2 == 0:
                                nc.scalar.activation(
                                    h1c, p1[:], AF.Relu, bias=b1s[:, mt : mt + 1]
                                )
                            else:
                                nc.vector.tensor_scalar(
                                    h1c,
                                    p1[:],
                                    b1s[:, mt : mt + 1],
                                    0.0,
                                    mybir.AluOpType.add,
                                    mybir.AluOpType.max,
                                )
                        # ---- layer 2 + fused layer 3 accumulation
                        p3 = ps3.tile([D, BT], F32, tag="p3")
                        hbs = []
                        for mt in range(MC):
                            p2 = ps12.tile([128, BT], F32, tag="p12")
                            for kt in range(MC):
                                nc.tensor.matmul(
                                    p2[:],
                                    w2[kt][:, mt * 128 : (mt + 1) * 128],
                                    h1[:, kt * BT : (kt + 1) * BT],
                                    start=(kt == 0),
                                    stop=(kt == MC - 1),
                                )
                            if mt == 8:
                                # open the L3 group: p3 = I @ s  (the Euler
                                # "+ s" term)
                                nc.tensor.matmul(
                                    p3[:],
                                    ident[:],
                                    sc,
                                    start=True,
                                    stop=False,
                                    skip_group_check=True,
                                )
                            if mt >= 8:
                                # layer-3 matmul for the chunk produced 8
                                # blocks ago; first 8 here, last 8 deferred
                                nc.tensor.matmul(
                                    p3[:],
                                    w3[mt - 8][:],
                                    hbs[mt - 8][:],
                                    start=False,
                                    stop=False,
                                    skip_group_check=True,
                                )
                            hb = h2p.tile([128, BT], F32R, tag="h2")
                            nc.scalar.activation(
                                hb[:], p2[:], AF.Relu, bias=b2s[:, mt : mt + 1]
                            )
                            hbs.append(hb)
                        pending = (
                            p3,
                            hbs[8:],
                            sc,
                            out_d.ap()[
                                step, :, (g0 + bt) * BT : (g0 + bt + 1) * BT
                            ],
                        )
                flush_pending()
                pending = None
    nc.compile()
    return nc


_NC_CACHE = {}


def _get_nc(b_shard):
    if b_shard not in _NC_CACHE:
        _NC_CACHE[b_shard] = build(b_shard)
    return _NC_CACHE[b_shard]


def _ntff_hook(so_path="/opt/axon/libaxon_pjrt.so"):
    """NTFF profiling via the axon .so (the antenv.axon_hooks shim is not
    installed in this image). Returns a contextmanager or None."""
    import ctypes, contextlib

    try:
        lib = ctypes.CDLL(so_path)
    except OSError:
        return None
    if not hasattr(lib, "axon_start_nrt_profile"):
        return None
    lib.axon_start_nrt_profile.argtypes = [
        ctypes.POINTER(ctypes.c_int64),
        ctypes.c_size_t,
    ]
    lib.axon_start_nrt_profile.restype = ctypes.c_int64
    lib.axon_stop_nrt_profile.argtypes = [ctypes.c_char_p]
    lib.axon_stop_nrt_profile.restype = ctypes.c_int64

    @contextlib.contextmanager
    def _hook(output_dir, device_ids):
        import jax

        jax.devices()
        if device_ids:
            ids = (ctypes.c_int64 * len(device_ids))(*device_ids)
            rc = lib.axon_start_nrt_profile(ids, len(device_ids))
        else:
            rc = lib.axon_start_nrt_profile(None, 0)
        if rc != 0:
            raise RuntimeError(f"axon_start_nrt_profile rc={rc}")
        try:
            yield
        finally:
            n = lib.axon_stop_nrt_profile(str(output_dir).encode())
            print(f"profile: {n} file(s) written to {output_dir}")

    return _hook


def kernel(x, z, W1, b1, W2, b2, W3, b3, _profile_dir=None, _profile_cores=(0,)):
    nc = _get_nc(B_SHARD)
    f32 = lambda a: np.ascontiguousarray(np.asarray(a, dtype=np.float32))
    x, z, W1, b1, W2, b2, W3, b3 = map(f32, (x, z, W1, b1, W2, b2, W3, b3))
    s0 = np.concatenate([x, z], axis=1)  # [B, 24]
    s0T = np.ascontiguousarray(s0.T)  # [24, B]
    W3s = np.ascontiguousarray(W3 * DT)
    b3s = np.ascontiguousarray(b3 * DT)
    in_maps = []
    for i in range(N_CORES):
        in_maps.append(
            {
                "sT": np.ascontiguousarray(s0T[:, i * B_SHARD : (i + 1) * B_SHARD]),
                "W1": W1,
                "W2": W2,
                "W3": W3s,
                "b1": b1,
                "b2": b2,
                "b3": b3s,
            }
        )
    if _profile_dir is not None:
        hook = _ntff_hook()
        if hook is None:
            raise RuntimeError("NTFF profiling hook unavailable")
        with hook(_profile_dir, list(_profile_cores)):
            res = run_bass_kernel_spmd(nc, in_maps, core_ids=list(range(N_CORES)))
    else:
        res = run_bass_kernel_spmd(nc, in_maps, core_ids=list(range(N_CORES)))
    shards = [res.results[i]["out"] for i in range(N_CORES)]  # [5, 24, 8192]
    steps = np.concatenate(shards, axis=2)  # [5, 24, B]
    traj = np.empty((BATCH, N_STEPS + 1, D), dtype=np.float32)
    traj[:, 0, :] = s0
    traj[:, 1:, :] = steps.transpose(2, 0, 1)
    traj[:, :, IN_DIM:] = np.clip(traj[:, :, IN_DIM:], -2.0, 2.0)
    final_state = traj[:, -1, :].copy()
    force = final_state[:, IN_DIM:].copy()
    return traj, final_state, force


# revision 13
# speedup vs baseline: 1.0015x; 1.0015x over previous
"""Trainium2 Bass kernel for nn_ActorNetwork (Euler-integrated 3-layer MLP).

reference semantics:
    s0 = concat(x, z)                       # [B, 24]
    5x Euler steps: s += 0.2 * MLP(s)       # MLP: 24 ->2048 relu ->2048 relu ->24
    traj = [s0, s1..s5] -> [B, 6, 24], force dims (15:) clamped to [-2, 2]
    returns (traj, final_state, force)

Strategy: pure data parallel over 8 NeuronCores (8192 samples each). All
activations are kept transposed ([feature, batch]) so the natural weight
layout is directly the matmul stationary operand and no transposes are
needed anywhere. Matmuls run as float32r (full-rate fp32 on the PE for
free dims >= 256). Layer biases + ReLU fold into ScalarE activations;
the Euler update is a VectorE add into PSUM followed by a ScalarE
round-copy back into the resident state tile.
"""

import sys

sys.path.insert(0, "/opt/trn_rl_repo")

import numpy as np
import concourse.bass as bass
import concourse.tile as tile
from concourse import bacc, masks, mybir
from concourse.bass_utils import run_bass_kernel_spmd

F32 = mybir.dt.float32
F32R = mybir.dt.float32r
AF = mybir.ActivationFunctionType

N_CORES = 8
BATCH = 65536
B_SHARD = BATCH // N_CORES  # 8192
IN_DIM = 15
FORCE_DIM = 9
D = IN_DIM + FORCE_DIM  # 24
H = 2048
MC = H // 128  # 16 hidden chunks
N_STEPS = 5
DT = 0.2
BT = 512  # batch tile (matmul moving free dim)
NBT = B_SHARD // BT  # 16
GROUP = 8  # batch tiles per resident state group
NG = NBT // GROUP  # 2


def build(b_shard=B_SHARD):
    nbt = b_shard // BT
    ngroups = max(1, nbt // GROUP)
    group = nbt // ngroups

    nc = bacc.Bacc("TRN2", target_bir_lowering=False, debug=False)
    sT_d = nc.dram_tensor("sT", [D, b_shard], F32R, kind="ExternalInput")
    W1_d = nc.dram_tensor("W1", [D, H], F32R, kind="ExternalInput")
    W2_d = nc.dram_tensor("W2", [H, H], F32R, kind="ExternalInput")
    W3_d = nc.dram_tensor("W3", [H, D], F32R, kind="ExternalInput")  # pre-scaled by DT
    b1_d = nc.dram_tensor("b1", [H], F32, kind="ExternalInput")
    b2_d = nc.dram_tensor("b2", [H], F32, kind="ExternalInput")
    b3_d = nc.dram_tensor("b3", [D], F32, kind="ExternalInput")  # pre-scaled by DT
    out_d = nc.dram_tensor("out", [N_STEPS, D, b_shard], F32, kind="ExternalOutput")

    with tile.TileContext(nc) as tc:
        with (
            tc.tile_pool(name="wpool", bufs=1) as wpool,
            tc.tile_pool(name="state", bufs=1) as spool,
            tc.tile_pool(name="h1p", bufs=1) as h1p,
            tc.tile_pool(name="h2p", bufs=10) as h2p,
            tc.tile_pool(name="ps12", bufs=6, space="PSUM") as ps12,
            tc.tile_pool(name="ps3", bufs=2, space="PSUM") as ps3,
        ):
            # --- weights / biases, resident for the whole kernel ---
            w1 = wpool.tile([D, H], F32R, tag="w1")
            nc.sync.dma_start(w1[:], W1_d.ap())
            w2 = []
            for kt in range(MC):
                t = wpool.tile([128, H], F32R, tag=f"w2_{kt}")
                nc.sync.dma_start(t[:], W2_d.ap()[kt * 128 : (kt + 1) * 128, :])
                w2.append(t)
            w3 = []
            for kt in range(MC):
                t = wpool.tile([128, D], F32R, tag=f"w3_{kt}")
                nc.sync.dma_start(t[:], W3_d.ap()[kt * 128 : (kt + 1) * 128, :])
                w3.append(t)
            b1s = wpool.tile([128, MC], F32, tag="b1s")
            nc.sync.dma_start(b1s[:], b1_d.ap().rearrange("(t p) -> p t", p=128))
            b2s = wpool.tile([128, MC], F32, tag="b2s")
            nc.sync.dma_start(b2s[:], b2_d.ap().rearrange("(t p) -> p t", p=128))
            b3s = wpool.tile([D, 1], F32, tag="b3s")
            nc.sync.dma_start(b3s[:], b3_d.ap().rearrange("(p o) -> p o", o=1))
            # 24x24 identity (f32r) - folds the Euler "+ s" into the L3
            # PSUM accumulation as one tiny matmul
            ident_f = wpool.tile([D, D], F32, tag="ident_f")
            masks.make_identity(nc, ident_f[:])
            ident = wpool.tile([D, D], F32R, tag="ident")
            nc.scalar.activation(ident[:], ident_f[:], AF.Identity, bias=0.0)

            h1 = h1p.tile([128, MC * BT], F32R, tag="h1")

            for g in range(ngroups):
                g0 = g * group
                # resident rounded state for this group's batch tiles
                scr = spool.tile([D, group * BT], F32R, tag="scr")
                nc.sync.dma_start(
                    scr[:], sT_d.ap()[:, g0 * BT : (g0 + group) * BT]
                )
                # pending: the previous batch tile's layer-3 tail (8
                # matmuls) + Euler epilogue, deferred into the next tile's
                # layer-1 phase so the PE has matmul work while the ReLU
                # (PSUM->SBUF) pipeline catches up.
                pending = None

                def flush_pending():
                    p3p, hbs, scp, outp = pending
                    for i in range(8):
                        nc.tensor.matmul(
                            p3p[:],
                            w3[8 + i][:],
                            hbs[i][:],
                            start=False,
                            stop=(i == 7),
                            skip_group_check=True,
                        )
                    # Euler epilogue: p3 already holds s + h2@(DT*W3);
                    # add DT*b3, round to f32r, store the new state
                    nc.scalar.activation(scp, p3p[:], AF.Identity, bias=b3s[:])
                    nc.sync.dma_start(outp, scp.bitcast(F32))

                for step in range(N_STEPS):
                    for bt in range(group):
                        sc = scr[:, bt * BT : (bt + 1) * BT]
                        # ---- layer 1: h1[mt] = relu(W1[:,mt].T @ s + b1[mt])
                        # bias+relu alternates ScalarE/VectorE so the L1
                        # epilogue keeps pace with the PE
                        for mt in range(MC):
                            p1 = ps12.tile([128, BT], F32, tag="p12")
                            nc.tensor.matmul(
                                p1[:],
                                w1[:, mt * 128 : (mt + 1) * 128],
                                sc,
                                start=True,
                                stop=True,
                            )
                            if pending is not None and mt == 5:
                                flush_pending()
                                pending = None
                            h1c = h1[:, mt * BT : (mt + 1) * BT]
                            if mt % 2 == 0:
                                nc.scalar.activation(
                                    h1c, p1[:], AF.Relu, bias=b1s[:, mt : mt + 1]
                                )
                            else:
                                nc.vector.tensor_scalar(
                                    h1c,
                                    p1[:],
                                    b1s[:, mt : mt + 1],
                                    0.0,
                                    mybir.AluOpType.add,
                                    mybir.AluOpType.max,
                                )
                        # ---- layer 2 + fused layer 3 accumulation
                        p3 = ps3.tile([D, BT], F32, tag="p3")
                        hbs = []
                        for mt in range(MC):
                            p2 = ps12.tile([128, BT], F32, tag="p12")
                            for kt in range(MC):
                                nc.tensor.matmul(
                                    p2[:],
                                    w2[kt][:, mt * 128 : (mt + 1) * 128],
                                    h1[:, kt * BT : (kt + 1) * BT],
                                    start=(kt == 0),
                                    stop=(kt == MC - 1),
                                )
                            if mt == 8:
                                # open the L3 group: p3 = I @ s  (the Euler
                                # "+ s" term)
                                nc.tensor.matmul(
                                    p3[:],
                                    ident[:],
                                    sc,
                                    start=True,
                                    stop=False,
                                    skip_group_check=True,
                                )
                            if mt >= 8:
                                # layer-3 matmul for the chunk produced 8
                                # blocks ago; first 8 here, last 8 deferred
                                nc.tensor.matmul(
                                    p3[:],
                                    w3[mt - 8][:],
                                    hbs[mt - 8][:],
                                    start=False,
                                    stop=False,
                                    skip_group_check=True,
                                )
                            hb = h2p.tile([128, BT], F32R, tag="h2")
                            nc.scalar.activation(
                                hb[:], p2[:], AF.Relu, bias=b2s[:, mt : mt + 1]
                            )
                            hbs.append(hb)
                        pending = (
                            p3,
                            hbs[8:],
                            sc,
                            out_d.ap()[
                                step, :, (g0 + bt) * BT : (g0 + bt + 1) * BT
                            ],
                        )
                flush_pending()
                pending = None
    nc.compile()
    return nc


_NC_CACHE = {}


def _get_nc(b_shard):
    if b_shard not in _NC_CACHE:
        _NC_CACHE[b_shard] = build(b_shard)
    return _NC_CACHE[b_shard]


def _ntff_hook(so_path="/opt/axon/libaxon_pjrt.so"):
    """NTFF profiling via the axon .so (the antenv.axon_hooks shim is not
    installed in this image). Returns a contextmanager or None."""
    import ctypes, contextlib

    try:
        lib = ctypes.CDLL(so_path)
    except OSError:
        return None
    if not hasattr(lib, "axon_start_nrt_profile"):
        return None
    lib.axon_start_nrt_profile.argtypes = [
        ctypes.POINTER(ctypes.c_int64),
        ctypes.c_size_t,
    ]
    lib.axon_start_nrt_profile.restype = ctypes.c_int64
    lib.axon_stop_nrt_profile.argtypes = [ctypes.c_char_p]
    lib.axon_stop_nrt_profile.restype = ctypes.c_int64

    @contextlib.contextmanager
    def _hook(output_dir, device_ids):
        import jax

        jax.devices()
        if device_ids:
            ids = (ctypes.c_int64 * len(device_ids))(*device_ids)
            rc = lib.axon_start_nrt_profile(ids, len(device_ids))
        else:
            rc = lib.axon_start_nrt_profile(None, 0)
        if rc != 0:
            raise RuntimeError(f"axon_start_nrt_profile rc={rc}")
        try:
            yield
        finally:
            n = lib.axon_stop_nrt_profile(str(output_dir).encode())
            print(f"profile: {n} file(s) written to {output_dir}")

    return _hook


def kernel(x, z, W1, b1, W2, b2, W3, b3, _profile_dir=None, _profile_cores=(0,)):
    nc = _get_nc(B_SHARD)
    f32 = lambda a: np.ascontiguousarray(np.asarray(a, dtype=np.float32))
    x, z, W1, b1, W2, b2, W3, b3 = map(f32, (x, z, W1, b1, W2, b2, W3, b3))
    s0 = np.concatenate([x, z], axis=1)  # [B, 24]
    s0T = np.ascontiguousarray(s0.T)  # [24, B]
    W3s = np.ascontiguousarray(W3 * DT)
    b3s = np.ascontiguousarray(b3 * DT)
    in_maps = []
    for i in range(N_CORES):
        in_maps.append(
            {
                "sT": np.ascontiguousarray(s0T[:, i * B_SHARD : (i + 1) * B_SHARD]),
                "W1": W1,
                "W2": W2,
                "W3": W3s,
                "b1": b1,
                "b2": b2,
                "b3": b3s,
            }
        )
    if _profile_dir is not None:
        hook = _ntff_hook()
        if hook is None:
            raise RuntimeError("NTFF profiling hook unavailable")
        with hook(_profile_dir, list(_profile_cores)):
            res = run_bass_kernel_spmd(nc, in_maps, core_ids=list(range(N_CORES)))
    else:
        res = run_bass_kernel_spmd(nc, in_maps, core_ids=list(range(N_CORES)))
    shards = [res.results[i]["out"] for i in range(N_CORES)]  # [5, 24, 8192]
    steps = np.concatenate(shards, axis=2)  # [5, 24, B]
    traj = np.empty((BATCH, N_STEPS + 1, D), dtype=np.float32)
    traj[:, 0, :] = s0
    traj[:, 1:, :] = steps.transpose(2, 0, 1)
    traj[:, :, IN_DIM:] = np.clip(traj[:, :, IN_DIM:], -2.0, 2.0)
    final_state = traj[:, -1, :].copy()
    force = final_state[:, IN_DIM:].copy()
    return traj, final_state, force


# revision 19
# speedup vs baseline: 1.0020x; 1.0005x over previous
"""Trainium2 Bass kernel for nn_ActorNetwork (Euler-integrated 3-layer MLP).

reference semantics:
    s0 = concat(x, z)                       # [B, 24]
    5x Euler steps: s += 0.2 * MLP(s)       # MLP: 24 ->2048 relu ->2048 relu ->24
    traj = [s0, s1..s5] -> [B, 6, 24], force dims (15:) clamped to [-2, 2]
    returns (traj, final_state, force)

Strategy: pure data parallel over 8 NeuronCores (8192 samples each). All
activations are kept transposed ([feature, batch]) so the natural weight
layout is directly the matmul stationary operand and no transposes are
needed anywhere. Matmuls run as float32r (full-rate fp32 on the PE for
free dims >= 256). Layer biases + ReLU fold into ScalarE activations;
the Euler update is a VectorE add into PSUM followed by a ScalarE
round-copy back into the resident state tile.
"""

import sys

sys.path.insert(0, "/opt/trn_rl_repo")

import numpy as np
import concourse.bass as bass
import concourse.tile as tile
from concourse import bacc, masks, mybir
from concourse.bass_utils import run_bass_kernel_spmd

F32 = mybir.dt.float32
F32R = mybir.dt.float32r
AF = mybir.ActivationFunctionType

N_CORES = 8
BATCH = 65536
B_SHARD = BATCH // N_CORES  # 8192
IN_DIM = 15
FORCE_DIM = 9
D = IN_DIM + FORCE_DIM  # 24
H = 2048
MC = H // 128  # 16 hidden chunks
N_STEPS = 5
DT = 0.2
BT = 512  # batch tile (matmul moving free dim)
NBT = B_SHARD // BT  # 16
GROUP = 8  # batch tiles per resident state group
NG = NBT // GROUP  # 2


def build(b_shard=B_SHARD):
    nbt = b_shard // BT
    ngroups = max(1, nbt // GROUP)
    group = nbt // ngroups

    nc = bacc.Bacc("TRN2", target_bir_lowering=False, debug=False)
    sT_d = nc.dram_tensor("sT", [D, b_shard], F32R, kind="ExternalInput")
    W1_d = nc.dram_tensor("W1", [D, H], F32R, kind="ExternalInput")
    W2_d = nc.dram_tensor("W2", [H, H], F32R, kind="ExternalInput")
    W3_d = nc.dram_tensor("W3", [H, D], F32R, kind="ExternalInput")  # pre-scaled by DT
    b1_d = nc.dram_tensor("b1", [H], F32, kind="ExternalInput")
    b2_d = nc.dram_tensor("b2", [H], F32, kind="ExternalInput")
    b3_d = nc.dram_tensor("b3", [D], F32, kind="ExternalInput")  # pre-scaled by DT
    out_d = nc.dram_tensor("out", [N_STEPS, D, b_shard], F32, kind="ExternalOutput")

    with tile.TileContext(nc) as tc:
        with (
            tc.tile_pool(name="wpool", bufs=1) as wpool,
            tc.tile_pool(name="state", bufs=1) as spool,
            tc.tile_pool(name="h1p", bufs=1) as h1p,
            tc.tile_pool(name="h2p", bufs=10) as h2p,
            tc.tile_pool(name="ps12", bufs=6, space="PSUM") as ps12,
            tc.tile_pool(name="ps3", bufs=2, space="PSUM") as ps3,
        ):
            # --- weights / biases, resident for the whole kernel ---
            w1 = wpool.tile([D, H], F32R, tag="w1")
            nc.sync.dma_start(w1[:], W1_d.ap())
            w2 = []
            for kt in range(MC):
                t = wpool.tile([128, H], F32R, tag=f"w2_{kt}")
                nc.sync.dma_start(t[:], W2_d.ap()[kt * 128 : (kt + 1) * 128, :])
                w2.append(t)
            w3 = []
            for kt in range(MC):
                t = wpool.tile([128, D], F32R, tag=f"w3_{kt}")
                nc.sync.dma_start(t[:], W3_d.ap()[kt * 128 : (kt + 1) * 128, :])
                w3.append(t)
            b1s = wpool.tile([128, MC], F32, tag="b1s")
            nc.sync.dma_start(b1s[:], b1_d.ap().rearrange("(t p) -> p t", p=128))
            b2s = wpool.tile([128, MC], F32, tag="b2s")
            nc.sync.dma_start(b2s[:], b2_d.ap().rearrange("(t p) -> p t", p=128))
            b3s = wpool.tile([D, 1], F32, tag="b3s")
            nc.sync.dma_start(b3s[:], b3_d.ap().rearrange("(p o) -> p o", o=1))
            # 24x24 identity (f32r) - folds the Euler "+ s" into the L3
            # PSUM accumulation as one tiny matmul
            ident_f = wpool.tile([D, D], F32, tag="ident_f")
            masks.make_identity(nc, ident_f[:])
            ident = wpool.tile([D, D], F32R, tag="ident")
            nc.scalar.activation(ident[:], ident_f[:], AF.Identity, bias=0.0)

            h1 = h1p.tile([128, MC * BT], F32R, tag="h1")

            for g in range(ngroups):
                g0 = g * group
                # resident rounded state for this group's batch tiles
                scr = spool.tile([D, group * BT], F32R, tag="scr")
                nc.sync.dma_start(
                    scr[:], sT_d.ap()[:, g0 * BT : (g0 + group) * BT]
                )
                # pending: the previous batch tile's layer-3 tail (8
                # matmuls) + Euler epilogue, deferred into the next tile's
                # layer-1 phase so the PE has matmul work while the ReLU
                # (PSUM->SBUF) pipeline catches up.
                pending = None

                def flush_pending():
                    p3p, hbs, scp, outp = pending
                    for i in range(8):
                        nc.tensor.matmul(
                            p3p[:],
                            w3[8 + i][:],
                            hbs[i][:],
                            start=False,
                            stop=(i == 7),
                            skip_group_check=True,
                        )
                    # Euler epilogue: p3 already holds s + h2@(DT*W3);
                    # add DT*b3, round to f32r, store the new state
                    nc.scalar.activation(scp, p3p[:], AF.Identity, bias=b3s[:])
                    nc.sync.dma_start(outp, scp.bitcast(F32))

                for step in range(N_STEPS):
                    for bt in range(group):
                        sc = scr[:, bt * BT : (bt + 1) * BT]
                        # ---- layer 1: h1[mt] = relu(W1[:,mt].T @ s + b1[mt])
                        # bias+relu alternates ScalarE/VectorE so the L1
                        # epilogue keeps pace with the PE
                        for mt in range(MC):
                            p1 = ps12.tile([128, BT], F32, tag="p12")
                            nc.tensor.matmul(
                                p1[:],
                                w1[:, mt * 128 : (mt + 1) * 128],
                                sc,
                                start=True,
                                stop=True,
                            )
                            if pending is not None and mt == 5:
                                flush_pending()
                                pending = None
                            h1c = h1[:, mt * BT : (mt + 1) * BT]
                            if mt % 2 == 0:
                                nc.scalar.activation(
                                    h1c, p1[:], AF.Relu, bias=b1s[:, mt : mt + 1]
                                )
                            else:
                                nc.vector.tensor_scalar(
                                    h1c,
                                    p1[:],
                                    b1s[:, mt : mt + 1],
                                    0.0,
                                    mybir.AluOpType.add,
                                    mybir.AluOpType.max,
                                )
                        # ---- layer 2 + fused layer 3 accumulation
                        p3 = ps3.tile([D, BT], F32, tag="p3")
                        hbs = []
                        for mt in range(MC):
                            p2 = ps12.tile([128, BT], F32, tag="p12")
                            for kt in range(MC):
                                nc.tensor.matmul(
                                    p2[:],
                                    w2[kt][:, mt * 128 : (mt + 1) * 128],
                                    h1[:, kt * BT : (kt + 1) * BT],
                                    start=(kt == 0),
                                    stop=(kt == MC - 1),
                                )
                            if mt == 8:
                                # open the L3 group: p3 = I @ s  (the Euler
                                # "+ s" term)
                                nc.tensor.matmul(
                                    p3[:],
                                    ident[:],
                                    sc,
                                    start=True,
                                    stop=False,
                                    skip_group_check=True,
                                )
                            if mt >= 8:
                                # layer-3 matmul for the chunk produced 8
                                # blocks ago; first 8 here, last 8 deferred
                                nc.tensor.matmul(
                                    p3[:],
                                    w3[mt - 8][:],
                                    hbs[mt - 8][:],
                                    start=False,
                                    stop=False,
                                    skip_group_check=True,
                                )
                            hb = h2p.tile([128, BT], F32R, tag="h2")
                            nc.scalar.activation(
                                hb[:], p2[:], AF.Relu, bias=b2s[:, mt : mt + 1]
                            )
                            hbs.append(hb)
                        pending = (
                            p3,
                            hbs[8:],
                            sc,
                            out_d.ap()[
                                step, :, (g0 + bt) * BT : (g0 + bt + 1) * BT
                            ],
                        )
                flush_pending()
                pending = None
    nc.compile()
    return nc


_NC_CACHE = {}


def _get_nc(b_shard):
    if b_shard not in _NC_CACHE:
        _NC_CACHE[b_shard] = build(b_shard)
    return _NC_CACHE[b_shard]


def _ntff_hook(so_path="/opt/axon/libaxon_pjrt.so"):
    """NTFF profiling via the axon .so (the antenv.axon_hooks shim is not
    installed in this image). Returns a contextmanager or None."""
    import ctypes, contextlib

    try:
        lib = ctypes.CDLL(so_path)
    except OSError:
        return None
    if not hasattr(lib, "axon_start_nrt_profile"):
        return None
    lib.axon_start_nrt_profile.argtypes = [
        ctypes.POINTER(ctypes.c_int64),
        ctypes.c_size_t,
    ]
    lib.axon_start_nrt_profile.restype = ctypes.c_int64
    lib.axon_stop_nrt_profile.argtypes = [ctypes.c_char_p]
    lib.axon_stop_nrt_profile.restype = ctypes.c_int64

    @contextlib.contextmanager
    def _hook(output_dir, device_ids):
        import jax

        jax.devices()
        if device_ids:
            ids = (ctypes.c_int64 * len(device_ids))(*device_ids)
            rc = lib.axon_start_nrt_profile(ids, len(device_ids))
        else:
            rc = lib.axon_start_nrt_profile(None, 0)
        if rc != 0:
            raise RuntimeError(f"axon_start_nrt_profile rc={rc}")
        try:
            yield
        finally:
            n = lib.axon_stop_nrt_profile(str(output_dir).encode())
            print(f"profile: {n} file(s) written to {output_dir}")

    return _hook


def kernel(x, z, W1, b1, W2, b2, W3, b3, _profile_dir=None, _profile_cores=(0,)):
    nc = _get_nc(B_SHARD)
    f32 = lambda a: np.ascontiguousarray(np.asarray(a, dtype=np.float32))
    x, z, W1, b1, W2, b2, W3, b3 = map(f32, (x, z, W1, b1, W2, b2, W3, b3))
    s0 = np.concatenate([x, z], axis=1)  # [B, 24]
    s0T = np.ascontiguousarray(s0.T)  # [24, B]
    W3s = np.ascontiguousarray(W3 * DT)
    b3s = np.ascontiguousarray(b3 * DT)
    in_maps = []
    for i in range(N_CORES):
        in_maps.append(
            {
                "sT": np.ascontiguousarray(s0T[:, i * B_SHARD : (i + 1) * B_SHARD]),
                "W1": W1,
                "W2": W2,
                "W3": W3s,
                "b1": b1,
                "b2": b2,
                "b3": b3s,
            }
        )
    if _profile_dir is not None:
        hook = _ntff_hook()
        if hook is None:
            raise RuntimeError("NTFF profiling hook unavailable")
        with hook(_profile_dir, list(_profile_cores)):
            res = run_bass_kernel_spmd(nc, in_maps, core_ids=list(range(N_CORES)))
    else:
        res = run_bass_kernel_spmd(nc, in_maps, core_ids=list(range(N_CORES)))
    shards = [res.results[i]["out"] for i in range(N_CORES)]  # [5, 24, 8192]
    steps = np.concatenate(shards, axis=2)  # [5, 24, B]
    traj = np.empty((BATCH, N_STEPS + 1, D), dtype=np.float32)
    traj[:, 0, :] = s0
    traj[:, 1:, :] = steps.transpose(2, 0, 1)
    traj[:, :, IN_DIM:] = np.clip(traj[:, :, IN_DIM:], -2.0, 2.0)
    final_state = traj[:, -1, :].copy()
    force = final_state[:, IN_DIM:].copy()
    return traj, final_state, force
